# revision 15
# baseline (speedup 1.0000x reference)
"""Causal GQA cross-attention kernel for Trainium2, 8-core SPMD.

Problem: q [2, 2048, 16, 128] f32, kv [2, 2048, 2, 8, 128] f32 ->
out [2, 2048, 16, 128] f32; causal mask (Sq == Sk), GQA with 2 q heads
per kv head, softmax scale 1/sqrt(128).

Sharding: 2 batches x 4 kv-head-pairs -> 8 cores. Each core gets 4 q
heads + 2 kv heads (its GQA groups), computes attention locally; no
collectives. Host splits/gathers.

Per-core algorithm, all matmuls bf16 (host pre-rounds to bf16 and
pre-transposes, so the device does zero transposes/casts):
  - Q^T/K^T [128d, S] bf16 loaded with contiguous DMAs.
  - V pre-augmented on host with a ones column: [128p, 16kb, 129] bf16;
    the ones column yields the softmax denominator for free in PV.
  - Work unit = (head, q-superblock sb of 512, k-block PAIR p covering
    kb = 2p, 2p+1) with 2p <= 4*sb+3 (block-causal). Per task:
      S^T[k, q] = (K^T kb-block).T @ Q^T[, sb]  -> PSUM [128, 2, 512],
      one matmul per half, q cols clipped to the causal range.
      P^T = exp(S^T * scale) in ONE activation over both halves
      (halves the scalar engine's per-instruction overhead); clipped to
      the pair's causal range; diagonal pairs masked multiplicatively
      on DVE with precomputed pair masks.
      PV: pv[j] += (P^T q-block j).T @ Vaug[kb]  (PSUM f32 accumulate).
  - Tasks are software-pipelined with lookahead 2: the tensor queue
    order is S(0) S(1) [S(2) PV(0)] [S(3) PV(1)] ... so exp on the
    scalar engine overlaps score/PV matmuls instead of stalling the PE.
  - Store unnormalized [q, 4*(129)] per (head, superblock); host
    divides by the denominator column and reorders.
"""

import math
import os
import sys

import ml_dtypes
import numpy as np

sys.path.insert(0, "/opt/trn_rl_repo")

import concourse.bass as bass  # noqa: E402
import concourse.mybir as mybir  # noqa: E402
import concourse.tile as tile  # noqa: E402
from concourse import bacc  # noqa: E402
from concourse.bass_utils import run_bass_kernel_spmd  # noqa: E402

B, SQ, SK, H, HKV, D = 2, 2048, 2048, 16, 8, 128
NCORES = 8
NQH = H * B // NCORES  # 4 q heads per core
NKVH = HKV * B // NCORES  # 2 kv heads per core
P = 128
NQB = SQ // P  # 16 q blocks of 128
NSB = 4  # q superblocks of 512
SBW = 512
NKB = SK // P  # 16 k blocks
SCALE = 1.0 / math.sqrt(D)
LOOKAHEAD = 2

F32 = mybir.dt.float32
BF16 = mybir.dt.bfloat16
BF16_NP = ml_dtypes.bfloat16

LAST_RESULTS = None
_CACHE = {}


def build_module():
    nc = bacc.Bacc(None, target_bir_lowering=False)

    q_d = nc.dram_tensor("q", [NQH, D, SQ], BF16, kind="ExternalInput")
    k_d = nc.dram_tensor("k", [NKVH, D, SK], BF16, kind="ExternalInput")
    v_d = nc.dram_tensor("v", [NKVH, P, NKB, D + 1], BF16, kind="ExternalInput")
    # per (head, superblock): row p holds the 4 q-block outputs concatenated
    o_d = nc.dram_tensor("o", [NQH, NSB, P, 4 * (D + 1)], F32, kind="ExternalOutput")

    heads = [(g, hl) for g in range(NKVH) for hl in range(2)]
    # flat task list: (head index, q superblock, k-block pair)
    tasks = []
    for hi in range(NQH):
        for sb in range(NSB):
            for p in range(2 * sb + 2):
                tasks.append((hi, sb, p))

    with tile.TileContext(nc) as tc:
        with (
            tc.tile_pool(name="const", bufs=1) as constp,
            tc.tile_pool(name="kt", bufs=2) as ktp,
            tc.tile_pool(name="qt", bufs=2) as qtp,
            tc.tile_pool(name="vaug", bufs=2) as vap,
            tc.tile_pool(name="pt", bufs=6) as ptp,
            tc.tile_pool(name="outs", bufs=4) as outp,
            tc.tile_pool(name="st", bufs=2, space="PSUM") as stp,
            tc.tile_pool(name="ppv", bufs=4, space="PSUM") as ppvp,
        ):
            # pair masks for r=0 (i=0) and r=2 (i=1):
            # msk[k, i, h, q] = 1 where q - k - 128*(2i+h) >= 0
            msk = constp.tile([P, 2, 2, SBW], BF16, tag="msk", name="msk")
            nc.gpsimd.memset(msk[:], 1.0)
            for i, r0 in enumerate((0, 2)):
                nc.gpsimd.affine_select(
                    out=msk[:, i],
                    in_=msk[:, i],
                    compare_op=mybir.AluOpType.is_ge,
                    fill=0.0,
                    base=-P * r0,
                    pattern=[[-P, 2], [1, SBW]],
                    channel_multiplier=-1,
                )

            kt_tiles, va_tiles, qt_tiles = {}, {}, {}

            def load_g(g):
                kt = ktp.tile([P, SK], BF16, tag="kt", name=f"kt{g}")
                nc.sync.dma_start(kt[:], k_d[g])
                va = vap.tile([P, NKB, D + 1], BF16, tag="va", name=f"va{g}")
                nc.sync.dma_start(va[:], v_d[g])
                kt_tiles[g] = kt
                va_tiles[g] = va

            def load_h(hi):
                qt = qtp.tile([P, SQ], BF16, tag="qt", name=f"qt{hi}")
                nc.sync.dma_start(qt[:], q_d[hi])
                qt_tiles[hi] = qt

            # first head's tensors arrive in consumption order, chunked so the
            # first score matmul only waits for the first slices; issue
            # alternates between the SP and gpsimd DGEs so two rings fill in
            # parallel
            kt0 = ktp.tile([P, SK], BF16, tag="kt", name="kt0")
            va0 = vap.tile([P, NKB, D + 1], BF16, tag="va", name="va0")
            qt0 = qtp.tile([P, SQ], BF16, tag="qt", name="qt0")
            kt_tiles[0] = kt0
            va_tiles[0] = va0
            qt_tiles[0] = qt0

            def kchunk(lo, hi):
                nc.sync.dma_start(kt0[:, lo * P : hi * P], k_d[0, :, lo * P : hi * P])

            def qchunk(sb):
                s = slice(sb * SBW, (sb + 1) * SBW)
                nc.sync.dma_start(qt0[:, s], q_d[0, :, s])

            def vchunk(lo, hi):
                nc.sync.dma_start(va0[:, lo:hi], v_d[0, :, lo:hi])

            kchunk(0, 2)
            qchunk(0)
            vchunk(0, 2)
            kchunk(2, 4)
            qchunk(1)
            vchunk(2, 4)
            kchunk(4, 8)
            qchunk(2)
            vchunk(4, 8)
            kchunk(8, 12)
            qchunk(3)
            vchunk(8, 16)
            kchunk(12, 16)

            pt_tiles = {}  # task idx -> pt AP
            pv_tiles = {}  # (hi, sb) -> [4 pv APs]
            ot_tiles = {}  # (hi, sb) -> staging AP

            def emit_s(t):
                hi, sb, p = tasks[t]
                g = heads[hi][0]
                r = 2 * p - 4 * sb
                st = stp.tile([P, 2, SBW], F32, tag="st", name=f"st{t}")
                for half in (0, 1):
                    kb = 2 * p + half
                    q_lo = max(0, kb - 4 * sb) * P
                    if t < 2:
                        q_lo = 0  # initialize the full PSUM slot on first use
                    nc.tensor.matmul(
                        st[:, half, q_lo:],
                        kt_tiles[g][:, kb * P : (kb + 1) * P],
                        qt_tiles[hi][:, sb * SBW + q_lo : (sb + 1) * SBW],
                        start=True,
                        stop=True,
                    )
                q_lo = max(0, r) * P
                pt = ptp.tile([P, 2, SBW], BF16, tag="pt", name=f"pt{t}")
                nc.scalar.activation(
                    pt[:, :, q_lo:],
                    st[:, :, q_lo:],
                    mybir.ActivationFunctionType.Exp,
                    scale=SCALE,
                )
                if r in (0, 2):
                    nc.vector.tensor_tensor(
                        out=pt[:, :, q_lo:],
                        in0=pt[:, :, q_lo:],
                        in1=msk[:, r // 2, :, q_lo:],
                        op=mybir.AluOpType.mult,
                    )
                pt_tiles[t] = pt

            def emit_pv(t):
                hi, sb, p = tasks[t]
                g = heads[hi][0]
                va = va_tiles[g]
                pt = pt_tiles.pop(t)
                if p == 0:
                    pv_tiles[(hi, sb)] = [
                        ppvp.tile([P, D + 1], F32, tag="pv", name=f"pv{hi}_{sb}_{j}")
                        for j in range(4)
                    ]
                    ot_tiles[(hi, sb)] = outp.tile(
                        [P, 4, D + 1], F32, tag="ot", name=f"ot{hi}_{sb}"
                    )
                pvs = pv_tiles[(hi, sb)]
                ot = ot_tiles[(hi, sb)]
                for half in (0, 1):
                    kb = 2 * p + half
                    for j in range(max(0, kb - 4 * sb), 4):
                        qb = 4 * sb + j
                        nc.tensor.matmul(
                            pvs[j][:],
                            pt[:, half, j * P : (j + 1) * P],
                            va[:, kb, :],
                            start=(kb == 0),
                            stop=(kb == qb),
                        )
                        if kb == qb:
                            nc.vector.tensor_copy(ot[:, j], pvs[j][:])
                if p == 2 * sb + 1:
                    nc.sync.dma_start(
                        o_d[hi, sb],
                        ot[:].rearrange("p j d -> p (j d)"),
                    )
                    del ot_tiles[(hi, sb)]
                    del pv_tiles[(hi, sb)]

            head_first_task = {}
            for t, (hi, sb, p) in enumerate(tasks):
                if hi not in head_first_task:
                    head_first_task[hi] = t

            for t in range(len(tasks) + LOOKAHEAD):
                if t < len(tasks):
                    hi = tasks[t][0]
                    if head_first_task.get(hi) == t:
                        # prefetch next head's tensors while this head runs
                        if hi + 1 < NQH:
                            load_h(hi + 1)
                        if hi == 1:
                            load_g(1)
                    emit_s(t)
                if t >= LOOKAHEAD:
                    emit_pv(t - LOOKAHEAD)

    nc.finalize()
    return nc


def _get_module():
    if "nc" not in _CACHE:
        _CACHE["nc"] = build_module()
    return _CACHE["nc"]


def kernel(q, kv):
    global LAST_RESULTS
    q = np.asarray(q, dtype=np.float32)
    kv = np.asarray(kv, dtype=np.float32)

    nc = _get_module()

    in_maps = []
    for c in range(NCORES):
        b, j = divmod(c, 4)
        # q^T: [4 heads, 128 d, 2048 s]
        q_bf = q[b][:, 4 * j : 4 * j + 4, :].astype(BF16_NP)  # [S, 4, D]
        q_t = np.ascontiguousarray(np.transpose(q_bf, (1, 2, 0)))
        # k^T: [2 kv heads, 128 d, 2048 s]
        k_bf = kv[b][:, 0, 2 * j : 2 * j + 2, :].astype(BF16_NP)  # [S, 2, D]
        k_t = np.ascontiguousarray(np.transpose(k_bf, (1, 2, 0)))
        # v augmented with ones col: [2, 128 p, 16 kb, 129]
        v_bf = kv[b][:, 1, 2 * j : 2 * j + 2, :].astype(BF16_NP)  # [S, 2, D]
        v_a = np.ones((NKVH, P, NKB, D + 1), BF16_NP)
        v_a[:, :, :, :D] = np.transpose(
            v_bf.reshape(NKB, P, NKVH, D), (2, 1, 0, 3)
        )
        in_maps.append({"q": q_t, "k": k_t, "v": v_a})

    trace = bool(int(os.environ.get("KERNEL_TRACE", "0")))
    kwargs = {}
    tdir = os.environ.get("KERNEL_TRACE_DIR")
    if tdir:
        kwargs["tmpdir"] = tdir
    res = run_bass_kernel_spmd(
        nc, in_maps, core_ids=list(range(NCORES)), trace=trace, **kwargs
    )
    LAST_RESULTS = res

    out = np.empty((B, SQ, H, D), np.float32)
    for c in range(NCORES):
        b, j = divmod(c, 4)
        o = res.results[c]["o"].reshape(NQH, NSB, P, 4, D + 1)
        o = np.transpose(o, (0, 1, 3, 2, 4)).reshape(NQH, SQ, D + 1)
        norm = o[..., :D] / o[..., D : D + 1]
        out[b, :, 4 * j : 4 * j + 4, :] = np.transpose(norm, (1, 0, 2))
    return out


# revision 16
# speedup vs baseline: 1.1795x; 1.1795x over previous
"""Causal GQA cross-attention kernel for Trainium2, 8-core SPMD.

Problem: q [2, 2048, 16, 128] f32, kv [2, 2048, 2, 8, 128] f32 ->
out [2, 2048, 16, 128] f32; causal mask (Sq == Sk), GQA with 2 q heads
per kv head, softmax scale 1/sqrt(128).

Sharding: 2 batches x 4 kv-head-pairs -> 8 cores. Each core gets 4 q
heads + 2 kv heads (its GQA groups), computes attention locally; no
collectives. Host splits/gathers.

Per-core algorithm, all matmuls bf16 (host pre-rounds to bf16 and
pre-transposes, so the device does zero transposes/casts):
  - Q^T/K^T [128d, S] bf16 loaded with contiguous DMAs.
  - V pre-augmented on host with a ones column: [128p, 16kb, 129] bf16;
    the ones column yields the softmax denominator for free in PV.
  - Work unit = (head, q-superblock sb of 512, k-block PAIR p covering
    kb = 2p, 2p+1) with 2p <= 4*sb+3 (block-causal). Per task:
      S^T[k, q] = (K^T kb-block).T @ Q^T[, sb]  -> PSUM [128, 2, 512],
      one matmul per half, q cols clipped to the causal range.
      P^T = exp(S^T * scale) in ONE activation over both halves
      (halves the scalar engine's per-instruction overhead); clipped to
      the pair's causal range; diagonal pairs masked multiplicatively
      on DVE with precomputed pair masks.
      PV: pv[j] += (P^T q-block j).T @ Vaug[kb]  (PSUM f32 accumulate).
  - Tasks are software-pipelined with lookahead 2: the tensor queue
    order is S(0) S(1) [S(2) PV(0)] [S(3) PV(1)] ... so exp on the
    scalar engine overlaps score/PV matmuls instead of stalling the PE.
  - Store unnormalized [q, 4*(129)] per (head, superblock); host
    divides by the denominator column and reorders.
"""

import math
import os
import sys

import ml_dtypes
import numpy as np

sys.path.insert(0, "/opt/trn_rl_repo")

import concourse.bass as bass  # noqa: E402
import concourse.mybir as mybir  # noqa: E402
import concourse.tile as tile  # noqa: E402
from concourse import bacc  # noqa: E402
from concourse.bass_utils import run_bass_kernel_spmd  # noqa: E402

B, SQ, SK, H, HKV, D = 2, 2048, 2048, 16, 8, 128
NCORES = 8
NQH = H * B // NCORES  # 4 q heads per core
NKVH = HKV * B // NCORES  # 2 kv heads per core
P = 128
NQB = SQ // P  # 16 q blocks of 128
NSB = 4  # q superblocks of 512
SBW = 512
NKB = SK // P  # 16 k blocks
SCALE = 1.0 / math.sqrt(D)
LOOKAHEAD = 2

F32 = mybir.dt.float32
BF16 = mybir.dt.bfloat16
BF16_NP = ml_dtypes.bfloat16

LAST_RESULTS = None
_CACHE = {}


def build_module():
    nc = bacc.Bacc(None, target_bir_lowering=False)

    q_d = nc.dram_tensor("q", [NQH, D, SQ], BF16, kind="ExternalInput")
    k_d = nc.dram_tensor("k", [NKVH, D, SK], BF16, kind="ExternalInput")
    v_d = nc.dram_tensor("v", [NKVH, P, NKB, D + 1], BF16, kind="ExternalInput")
    # per (head, superblock): row p holds the 4 q-block outputs concatenated
    o_d = nc.dram_tensor("o", [NQH, NSB, P, 4 * (D + 1)], F32, kind="ExternalOutput")

    heads = [(g, hl) for g in range(NKVH) for hl in range(2)]
    # flat task list: (head index, q superblock, k-block pair)
    tasks = []
    for hi in range(NQH):
        for sb in range(NSB):
            for p in range(2 * sb + 2):
                tasks.append((hi, sb, p))

    with tile.TileContext(nc) as tc:
        with (
            tc.tile_pool(name="const", bufs=1) as constp,
            tc.tile_pool(name="kt", bufs=2) as ktp,
            tc.tile_pool(name="qt", bufs=2) as qtp,
            tc.tile_pool(name="vaug", bufs=2) as vap,
            tc.tile_pool(name="pt", bufs=4) as ptp,
            tc.tile_pool(name="outs", bufs=4) as outp,
            tc.tile_pool(name="st", bufs=2, space="PSUM") as stp,
            tc.tile_pool(name="ppv", bufs=4, space="PSUM") as ppvp,
        ):
            # pair masks for r=0 (i=0) and r=2 (i=1):
            # msk[k, i, h, q] = 1 where q - k - 128*(2i+h) >= 0
            msk = constp.tile([P, 2, 2, SBW], BF16, tag="msk", name="msk")
            nc.gpsimd.memset(msk[:], 1.0)
            for i, r0 in enumerate((0, 2)):
                nc.gpsimd.affine_select(
                    out=msk[:, i],
                    in_=msk[:, i],
                    compare_op=mybir.AluOpType.is_ge,
                    fill=0.0,
                    base=-P * r0,
                    pattern=[[-P, 2], [1, SBW]],
                    channel_multiplier=-1,
                )

            kt_tiles, va_tiles, qt_tiles = {}, {}, {}

            def load_g(g):
                kt = ktp.tile([P, SK], BF16, tag="kt", name=f"kt{g}")
                nc.sync.dma_start(kt[:], k_d[g])
                va = vap.tile([P, NKB, D + 1], BF16, tag="va", name=f"va{g}")
                nc.sync.dma_start(va[:], v_d[g])
                kt_tiles[g] = kt
                va_tiles[g] = va

            def load_h(hi):
                qt = qtp.tile([P, SQ], BF16, tag="qt", name=f"qt{hi}")
                nc.sync.dma_start(qt[:], q_d[hi])
                qt_tiles[hi] = qt

            # first head's tensors arrive in consumption order, chunked so the
            # first score matmul only waits for the first slices; issue
            # alternates between the SP and gpsimd DGEs so two rings fill in
            # parallel
            kt0 = ktp.tile([P, SK], BF16, tag="kt", name="kt0")
            va0 = vap.tile([P, NKB, D + 1], BF16, tag="va", name="va0")
            qt0 = qtp.tile([P, SQ], BF16, tag="qt", name="qt0")
            kt_tiles[0] = kt0
            va_tiles[0] = va0
            qt_tiles[0] = qt0

            def kchunk(lo, hi):
                nc.sync.dma_start(kt0[:, lo * P : hi * P], k_d[0, :, lo * P : hi * P])

            def qchunk(sb):
                s = slice(sb * SBW, (sb + 1) * SBW)
                nc.sync.dma_start(qt0[:, s], q_d[0, :, s])

            def vchunk(lo, hi):
                nc.sync.dma_start(va0[:, lo:hi], v_d[0, :, lo:hi])

            kchunk(0, 2)
            qchunk(0)
            vchunk(0, 2)
            kchunk(2, 4)
            qchunk(1)
            vchunk(2, 4)
            kchunk(4, 8)
            qchunk(2)
            vchunk(4, 8)
            kchunk(8, 12)
            qchunk(3)
            vchunk(8, 16)
            kchunk(12, 16)

            pt_tiles = {}  # task idx -> pt AP
            pv_tiles = {}  # (hi, sb) -> [4 pv APs]
            ot_tiles = {}  # (hi, sb) -> staging AP

            def emit_s(t):
                hi, sb, p = tasks[t]
                g = heads[hi][0]
                r = 2 * p - 4 * sb
                st = stp.tile([P, 2, SBW], F32, tag="st", name=f"st{t}")
                for half in (0, 1):
                    kb = 2 * p + half
                    q_lo = max(0, kb - 4 * sb) * P
                    if t < 2:
                        q_lo = 0  # initialize the full PSUM slot on first use
                    nc.tensor.matmul(
                        st[:, half, q_lo:],
                        kt_tiles[g][:, kb * P : (kb + 1) * P],
                        qt_tiles[hi][:, sb * SBW + q_lo : (sb + 1) * SBW],
                        start=True,
                        stop=True,
                    )
                q_lo = max(0, r) * P
                pt = ptp.tile([P, 2, SBW], BF16, tag="pt", name=f"pt{t}")
                nc.scalar.activation(
                    pt[:, :, q_lo:],
                    st[:, :, q_lo:],
                    mybir.ActivationFunctionType.Exp,
                    scale=SCALE,
                )
                if r in (0, 2):
                    nc.vector.tensor_tensor(
                        out=pt[:, :, q_lo:],
                        in0=pt[:, :, q_lo:],
                        in1=msk[:, r // 2, :, q_lo:],
                        op=mybir.AluOpType.mult,
                    )
                pt_tiles[t] = pt

            def emit_pv(t):
                hi, sb, p = tasks[t]
                g = heads[hi][0]
                va = va_tiles[g]
                pt = pt_tiles.pop(t)
                if p == 0:
                    pv_tiles[(hi, sb)] = [
                        ppvp.tile([P, D + 1], F32, tag="pv", name=f"pv{hi}_{sb}_{j}")
                        for j in range(4)
                    ]
                    ot_tiles[(hi, sb)] = outp.tile(
                        [P, 4, D + 1], F32, tag="ot", name=f"ot{hi}_{sb}"
                    )
                pvs = pv_tiles[(hi, sb)]
                ot = ot_tiles[(hi, sb)]
                for half in (0, 1):
                    kb = 2 * p + half
                    for j in range(max(0, kb - 4 * sb), 4):
                        qb = 4 * sb + j
                        nc.tensor.matmul(
                            pvs[j][:],
                            pt[:, half, j * P : (j + 1) * P],
                            va[:, kb, :],
                            start=(kb == 0),
                            stop=(kb == qb),
                        )
                        if kb == qb:
                            nc.vector.tensor_copy(ot[:, j], pvs[j][:])
                if p == 2 * sb + 1:
                    nc.sync.dma_start(
                        o_d[hi, sb],
                        ot[:].rearrange("p j d -> p (j d)"),
                    )
                    del ot_tiles[(hi, sb)]
                    del pv_tiles[(hi, sb)]

            head_first_task = {}
            for t, (hi, sb, p) in enumerate(tasks):
                if hi not in head_first_task:
                    head_first_task[hi] = t

            for t in range(len(tasks) + LOOKAHEAD):
                if t < len(tasks):
                    hi = tasks[t][0]
                    if head_first_task.get(hi) == t:
                        # prefetch next head's tensors while this head runs
                        if hi + 1 < NQH:
                            load_h(hi + 1)
                        if hi == 1:
                            load_g(1)
                    emit_s(t)
                if t >= LOOKAHEAD:
                    emit_pv(t - LOOKAHEAD)

    nc.finalize()
    return nc


def _get_module():
    if "nc" not in _CACHE:
        _CACHE["nc"] = build_module()
    return _CACHE["nc"]


def kernel(q, kv):
    global LAST_RESULTS
    q = np.asarray(q, dtype=np.float32)
    kv = np.asarray(kv, dtype=np.float32)

    nc = _get_module()

    in_maps = []
    for c in range(NCORES):
        b, j = divmod(c, 4)
        # q^T: [4 heads, 128 d, 2048 s]
        q_bf = q[b][:, 4 * j : 4 * j + 4, :].astype(BF16_NP)  # [S, 4, D]
        q_t = np.ascontiguousarray(np.transpose(q_bf, (1, 2, 0)))
        # k^T: [2 kv heads, 128 d, 2048 s]
        k_bf = kv[b][:, 0, 2 * j : 2 * j + 2, :].astype(BF16_NP)  # [S, 2, D]
        k_t = np.ascontiguousarray(np.transpose(k_bf, (1, 2, 0)))
        # v augmented with ones col: [2, 128 p, 16 kb, 129]
        v_bf = kv[b][:, 1, 2 * j : 2 * j + 2, :].astype(BF16_NP)  # [S, 2, D]
        v_a = np.ones((NKVH, P, NKB, D + 1), BF16_NP)
        v_a[:, :, :, :D] = np.transpose(
            v_bf.reshape(NKB, P, NKVH, D), (2, 1, 0, 3)
        )
        in_maps.append({"q": q_t, "k": k_t, "v": v_a})

    trace = bool(int(os.environ.get("KERNEL_TRACE", "0")))
    kwargs = {}
    tdir = os.environ.get("KERNEL_TRACE_DIR")
    if tdir:
        kwargs["tmpdir"] = tdir
    res = run_bass_kernel_spmd(
        nc, in_maps, core_ids=list(range(NCORES)), trace=trace, **kwargs
    )
    LAST_RESULTS = res

    out = np.empty((B, SQ, H, D), np.float32)
    for c in range(NCORES):
        b, j = divmod(c, 4)
        o = res.results[c]["o"].reshape(NQH, NSB, P, 4, D + 1)
        o = np.transpose(o, (0, 1, 3, 2, 4)).reshape(NQH, SQ, D + 1)
        norm = o[..., :D] / o[..., D : D + 1]
        out[b, :, 4 * j : 4 * j + 4, :] = np.transpose(norm, (1, 0, 2))
    return out


# revision 17
# speedup vs baseline: 1.1815x; 1.0017x over previous
"""Causal GQA cross-attention kernel for Trainium2, 8-core SPMD.

Problem: q [2, 2048, 16, 128] f32, kv [2, 2048, 2, 8, 128] f32 ->
out [2, 2048, 16, 128] f32; causal mask (Sq == Sk), GQA with 2 q heads
per kv head, softmax scale 1/sqrt(128).

Sharding: 2 batches x 4 kv-head-pairs -> 8 cores. Each core gets 4 q
heads + 2 kv heads (its GQA groups), computes attention locally; no
collectives. Host splits/gathers.

Per-core algorithm, all matmuls bf16 (host pre-rounds to bf16 and
pre-transposes, so the device does zero transposes/casts):
  - Q^T/K^T [128d, S] bf16 loaded with contiguous DMAs.
  - V pre-augmented on host with a ones column: [128p, 16kb, 129] bf16;
    the ones column yields the softmax denominator for free in PV.
  - Work unit = (head, q-superblock sb of 512, k-block PAIR p covering
    kb = 2p, 2p+1) with 2p <= 4*sb+3 (block-causal). Per task:
      S^T[k, q] = (K^T kb-block).T @ Q^T[, sb]  -> PSUM [128, 2, 512],
      one matmul per half, q cols clipped to the causal range.
      P^T = exp(S^T * scale) in ONE activation over both halves
      (halves the scalar engine's per-instruction overhead); clipped to
      the pair's causal range; diagonal pairs masked multiplicatively
      on DVE with precomputed pair masks.
      PV: pv[j] += (P^T q-block j).T @ Vaug[kb]  (PSUM f32 accumulate).
  - Tasks are software-pipelined with lookahead 2: the tensor queue
    order is S(0) S(1) [S(2) PV(0)] [S(3) PV(1)] ... so exp on the
    scalar engine overlaps score/PV matmuls instead of stalling the PE.
  - Store unnormalized [q, 4*(129)] per (head, superblock); host
    divides by the denominator column and reorders.
"""

import math
import os
import sys

import ml_dtypes
import numpy as np

sys.path.insert(0, "/opt/trn_rl_repo")

import concourse.bass as bass  # noqa: E402
import concourse.mybir as mybir  # noqa: E402
import concourse.tile as tile  # noqa: E402
from concourse import bacc  # noqa: E402
from concourse.bass_utils import run_bass_kernel_spmd  # noqa: E402

B, SQ, SK, H, HKV, D = 2, 2048, 2048, 16, 8, 128
NCORES = 8
NQH = H * B // NCORES  # 4 q heads per core
NKVH = HKV * B // NCORES  # 2 kv heads per core
P = 128
NQB = SQ // P  # 16 q blocks of 128
NSB = 4  # q superblocks of 512
SBW = 512
NKB = SK // P  # 16 k blocks
SCALE = 1.0 / math.sqrt(D)
LOOKAHEAD = 2

F32 = mybir.dt.float32
BF16 = mybir.dt.bfloat16
BF16_NP = ml_dtypes.bfloat16

LAST_RESULTS = None
_CACHE = {}


def build_module():
    nc = bacc.Bacc(None, target_bir_lowering=False)

    q_d = nc.dram_tensor("q", [NQH, D, SQ], BF16, kind="ExternalInput")
    k_d = nc.dram_tensor("k", [NKVH, D, SK], BF16, kind="ExternalInput")
    v_d = nc.dram_tensor("v", [NKVH, P, NKB, D + 1], BF16, kind="ExternalInput")
    # per (head, superblock): row p holds the 4 q-block outputs concatenated
    o_d = nc.dram_tensor("o", [NQH, NSB, P, 4 * (D + 1)], F32, kind="ExternalOutput")

    heads = [(g, hl) for g in range(NKVH) for hl in range(2)]
    # flat task list: (head index, q superblock, k-block pair)
    tasks = []
    for hi in range(NQH):
        for sb in range(NSB):
            for p in range(2 * sb + 2):
                tasks.append((hi, sb, p))

    with tile.TileContext(nc) as tc:
        with (
            tc.tile_pool(name="const", bufs=1) as constp,
            tc.tile_pool(name="kt", bufs=2) as ktp,
            tc.tile_pool(name="qt", bufs=2) as qtp,
            tc.tile_pool(name="vaug", bufs=2) as vap,
            tc.tile_pool(name="pt", bufs=3) as ptp,
            tc.tile_pool(name="outs", bufs=4) as outp,
            tc.tile_pool(name="st", bufs=2, space="PSUM") as stp,
            tc.tile_pool(name="ppv", bufs=4, space="PSUM") as ppvp,
        ):
            # pair masks for r=0 (i=0) and r=2 (i=1):
            # msk[k, i, h, q] = 1 where q - k - 128*(2i+h) >= 0
            msk = constp.tile([P, 2, 2, SBW], BF16, tag="msk", name="msk")
            nc.gpsimd.memset(msk[:], 1.0)
            for i, r0 in enumerate((0, 2)):
                nc.gpsimd.affine_select(
                    out=msk[:, i],
                    in_=msk[:, i],
                    compare_op=mybir.AluOpType.is_ge,
                    fill=0.0,
                    base=-P * r0,
                    pattern=[[-P, 2], [1, SBW]],
                    channel_multiplier=-1,
                )

            kt_tiles, va_tiles, qt_tiles = {}, {}, {}

            def load_g(g):
                kt = ktp.tile([P, SK], BF16, tag="kt", name=f"kt{g}")
                nc.sync.dma_start(kt[:], k_d[g])
                va = vap.tile([P, NKB, D + 1], BF16, tag="va", name=f"va{g}")
                nc.sync.dma_start(va[:], v_d[g])
                kt_tiles[g] = kt
                va_tiles[g] = va

            def load_h(hi):
                qt = qtp.tile([P, SQ], BF16, tag="qt", name=f"qt{hi}")
                nc.sync.dma_start(qt[:], q_d[hi])
                qt_tiles[hi] = qt

            # first head's tensors arrive in consumption order, chunked so the
            # first score matmul only waits for the first slices; issue
            # alternates between the SP and gpsimd DGEs so two rings fill in
            # parallel
            kt0 = ktp.tile([P, SK], BF16, tag="kt", name="kt0")
            va0 = vap.tile([P, NKB, D + 1], BF16, tag="va", name="va0")
            qt0 = qtp.tile([P, SQ], BF16, tag="qt", name="qt0")
            kt_tiles[0] = kt0
            va_tiles[0] = va0
            qt_tiles[0] = qt0

            def kchunk(lo, hi):
                nc.sync.dma_start(kt0[:, lo * P : hi * P], k_d[0, :, lo * P : hi * P])

            def qchunk(sb):
                s = slice(sb * SBW, (sb + 1) * SBW)
                nc.sync.dma_start(qt0[:, s], q_d[0, :, s])

            def vchunk(lo, hi):
                nc.sync.dma_start(va0[:, lo:hi], v_d[0, :, lo:hi])

            kchunk(0, 2)
            qchunk(0)
            vchunk(0, 2)
            kchunk(2, 4)
            qchunk(1)
            vchunk(2, 4)
            kchunk(4, 8)
            qchunk(2)
            vchunk(4, 8)
            kchunk(8, 12)
            qchunk(3)
            vchunk(8, 16)
            kchunk(12, 16)

            pt_tiles = {}  # task idx -> pt AP
            pv_tiles = {}  # (hi, sb) -> [4 pv APs]
            ot_tiles = {}  # (hi, sb) -> staging AP

            def emit_s(t):
                hi, sb, p = tasks[t]
                g = heads[hi][0]
                r = 2 * p - 4 * sb
                st = stp.tile([P, 2, SBW], F32, tag="st", name=f"st{t}")
                for half in (0, 1):
                    kb = 2 * p + half
                    q_lo = max(0, kb - 4 * sb) * P
                    if t < 2:
                        q_lo = 0  # initialize the full PSUM slot on first use
                    nc.tensor.matmul(
                        st[:, half, q_lo:],
                        kt_tiles[g][:, kb * P : (kb + 1) * P],
                        qt_tiles[hi][:, sb * SBW + q_lo : (sb + 1) * SBW],
                        start=True,
                        stop=True,
                    )
                q_lo = max(0, r) * P
                pt = ptp.tile([P, 2, SBW], BF16, tag="pt", name=f"pt{t}")
                nc.scalar.activation(
                    pt[:, :, q_lo:],
                    st[:, :, q_lo:],
                    mybir.ActivationFunctionType.Exp,
                    scale=SCALE,
                )
                if r in (0, 2):
                    nc.vector.tensor_tensor(
                        out=pt[:, :, q_lo:],
                        in0=pt[:, :, q_lo:],
                        in1=msk[:, r // 2, :, q_lo:],
                        op=mybir.AluOpType.mult,
                    )
                pt_tiles[t] = pt

            def emit_pv(t):
                hi, sb, p = tasks[t]
                g = heads[hi][0]
                va = va_tiles[g]
                pt = pt_tiles.pop(t)
                if p == 0:
                    pv_tiles[(hi, sb)] = [
                        ppvp.tile([P, D + 1], F32, tag="pv", name=f"pv{hi}_{sb}_{j}")
                        for j in range(4)
                    ]
                    ot_tiles[(hi, sb)] = outp.tile(
                        [P, 4, D + 1], F32, tag="ot", name=f"ot{hi}_{sb}"
                    )
                pvs = pv_tiles[(hi, sb)]
                ot = ot_tiles[(hi, sb)]
                for half in (0, 1):
                    kb = 2 * p + half
                    for j in range(max(0, kb - 4 * sb), 4):
                        qb = 4 * sb + j
                        nc.tensor.matmul(
                            pvs[j][:],
                            pt[:, half, j * P : (j + 1) * P],
                            va[:, kb, :],
                            start=(kb == 0),
                            stop=(kb == qb),
                        )
                        if kb == qb:
                            nc.vector.tensor_copy(ot[:, j], pvs[j][:])
                if p == 2 * sb + 1:
                    nc.sync.dma_start(
                        o_d[hi, sb],
                        ot[:].rearrange("p j d -> p (j d)"),
                    )
                    del ot_tiles[(hi, sb)]
                    del pv_tiles[(hi, sb)]

            head_first_task = {}
            for t, (hi, sb, p) in enumerate(tasks):
                if hi not in head_first_task:
                    head_first_task[hi] = t

            for t in range(len(tasks) + LOOKAHEAD):
                if t < len(tasks):
                    hi = tasks[t][0]
                    if head_first_task.get(hi) == t:
                        # prefetch next head's tensors while this head runs
                        if hi + 1 < NQH:
                            load_h(hi + 1)
                        if hi == 1:
                            load_g(1)
                    emit_s(t)
                if t >= LOOKAHEAD:
                    emit_pv(t - LOOKAHEAD)

    nc.finalize()
    return nc


def _get_module():
    if "nc" not in _CACHE:
        _CACHE["nc"] = build_module()
    return _CACHE["nc"]


def kernel(q, kv):
    global LAST_RESULTS
    q = np.asarray(q, dtype=np.float32)
    kv = np.asarray(kv, dtype=np.float32)

    nc = _get_module()

    in_maps = []
    for c in range(NCORES):
        b, j = divmod(c, 4)
        # q^T: [4 heads, 128 d, 2048 s]
        q_bf = q[b][:, 4 * j : 4 * j + 4, :].astype(BF16_NP)  # [S, 4, D]
        q_t = np.ascontiguousarray(np.transpose(q_bf, (1, 2, 0)))
        # k^T: [2 kv heads, 128 d, 2048 s]
        k_bf = kv[b][:, 0, 2 * j : 2 * j + 2, :].astype(BF16_NP)  # [S, 2, D]
        k_t = np.ascontiguousarray(np.transpose(k_bf, (1, 2, 0)))
        # v augmented with ones col: [2, 128 p, 16 kb, 129]
        v_bf = kv[b][:, 1, 2 * j : 2 * j + 2, :].astype(BF16_NP)  # [S, 2, D]
        v_a = np.ones((NKVH, P, NKB, D + 1), BF16_NP)
        v_a[:, :, :, :D] = np.transpose(
            v_bf.reshape(NKB, P, NKVH, D), (2, 1, 0, 3)
        )
        in_maps.append({"q": q_t, "k": k_t, "v": v_a})

    trace = bool(int(os.environ.get("KERNEL_TRACE", "0")))
    kwargs = {}
    tdir = os.environ.get("KERNEL_TRACE_DIR")
    if tdir:
        kwargs["tmpdir"] = tdir
    res = run_bass_kernel_spmd(
        nc, in_maps, core_ids=list(range(NCORES)), trace=trace, **kwargs
    )
    LAST_RESULTS = res

    out = np.empty((B, SQ, H, D), np.float32)
    for c in range(NCORES):
        b, j = divmod(c, 4)
        o = res.results[c]["o"].reshape(NQH, NSB, P, 4, D + 1)
        o = np.transpose(o, (0, 1, 3, 2, 4)).reshape(NQH, SQ, D + 1)
        norm = o[..., :D] / o[..., D : D + 1]
        out[b, :, 4 * j : 4 * j + 4, :] = np.transpose(norm, (1, 0, 2))
    return out


# revision 18
# speedup vs baseline: 1.1833x; 1.0015x over previous
"""Causal GQA cross-attention kernel for Trainium2, 8-core SPMD.

Problem: q [2, 2048, 16, 128] f32, kv [2, 2048, 2, 8, 128] f32 ->
out [2, 2048, 16, 128] f32; causal mask (Sq == Sk), GQA with 2 q heads
per kv head, softmax scale 1/sqrt(128).

Sharding: 2 batches x 4 kv-head-pairs -> 8 cores. Each core gets 4 q
heads + 2 kv heads (its GQA groups), computes attention locally; no
collectives. Host splits/gathers.

Per-core algorithm, all matmuls bf16 (host pre-rounds to bf16 and
pre-transposes, so the device does zero transposes/casts):
  - Q^T/K^T [128d, S] bf16 loaded with contiguous DMAs.
  - V pre-augmented on host with a ones column: [128p, 16kb, 129] bf16;
    the ones column yields the softmax denominator for free in PV.
  - Work unit = (head, q-superblock sb of 512, k-block PAIR p covering
    kb = 2p, 2p+1) with 2p <= 4*sb+3 (block-causal). Per task:
      S^T[k, q] = (K^T kb-block).T @ Q^T[, sb]  -> PSUM [128, 2, 512],
      one matmul per half, q cols clipped to the causal range.
      P^T = exp(S^T * scale) in ONE activation over both halves
      (halves the scalar engine's per-instruction overhead); clipped to
      the pair's causal range; diagonal pairs masked multiplicatively
      on DVE with precomputed pair masks.
      PV: pv[j] += (P^T q-block j).T @ Vaug[kb]  (PSUM f32 accumulate).
  - Tasks are software-pipelined with lookahead 2: the tensor queue
    order is S(0) S(1) [S(2) PV(0)] [S(3) PV(1)] ... so exp on the
    scalar engine overlaps score/PV matmuls instead of stalling the PE.
  - Store unnormalized [q, 4*(129)] per (head, superblock); host
    divides by the denominator column and reorders.
"""

import math
import os
import sys

import ml_dtypes
import numpy as np

sys.path.insert(0, "/opt/trn_rl_repo")

import concourse.bass as bass  # noqa: E402
import concourse.mybir as mybir  # noqa: E402
import concourse.tile as tile  # noqa: E402
from concourse import bacc  # noqa: E402
from concourse.bass_utils import run_bass_kernel_spmd  # noqa: E402

B, SQ, SK, H, HKV, D = 2, 2048, 2048, 16, 8, 128
NCORES = 8
NQH = H * B // NCORES  # 4 q heads per core
NKVH = HKV * B // NCORES  # 2 kv heads per core
P = 128
NQB = SQ // P  # 16 q blocks of 128
NSB = 4  # q superblocks of 512
SBW = 512
NKB = SK // P  # 16 k blocks
SCALE = 1.0 / math.sqrt(D)
LOOKAHEAD = 2

F32 = mybir.dt.float32
BF16 = mybir.dt.bfloat16
BF16_NP = ml_dtypes.bfloat16

LAST_RESULTS = None
_CACHE = {}


def build_module():
    nc = bacc.Bacc(None, target_bir_lowering=False)

    q_d = nc.dram_tensor("q", [NQH, D, SQ], BF16, kind="ExternalInput")
    k_d = nc.dram_tensor("k", [NKVH, D, SK], BF16, kind="ExternalInput")
    v_d = nc.dram_tensor("v", [NKVH, P, NKB, D + 1], BF16, kind="ExternalInput")
    # per (head, superblock): row p holds the 4 q-block outputs concatenated
    o_d = nc.dram_tensor("o", [NQH, NSB, P, 4 * (D + 1)], F32, kind="ExternalOutput")

    heads = [(g, hl) for g in range(NKVH) for hl in range(2)]
    # flat task list: (head index, q superblock, k-block pair)
    tasks = []
    for hi in range(NQH):
        for sb in range(NSB):
            for p in range(2 * sb + 2):
                tasks.append((hi, sb, p))

    with tile.TileContext(nc) as tc:
        with (
            tc.tile_pool(name="const", bufs=1) as constp,
            tc.tile_pool(name="kt", bufs=2) as ktp,
            tc.tile_pool(name="qt", bufs=2) as qtp,
            tc.tile_pool(name="vaug", bufs=2) as vap,
            tc.tile_pool(name="pt", bufs=4) as ptp,
            tc.tile_pool(name="outs", bufs=4) as outp,
            tc.tile_pool(name="st", bufs=2, space="PSUM") as stp,
            tc.tile_pool(name="ppv", bufs=4, space="PSUM") as ppvp,
        ):
            # pair masks for r=0 (i=0) and r=2 (i=1):
            # msk[k, i, h, q] = 1 where q - k - 128*(2i+h) >= 0
            msk = constp.tile([P, 2, 2, SBW], BF16, tag="msk", name="msk")
            nc.gpsimd.memset(msk[:], 1.0)
            for i, r0 in enumerate((0, 2)):
                nc.gpsimd.affine_select(
                    out=msk[:, i],
                    in_=msk[:, i],
                    compare_op=mybir.AluOpType.is_ge,
                    fill=0.0,
                    base=-P * r0,
                    pattern=[[-P, 2], [1, SBW]],
                    channel_multiplier=-1,
                )

            kt_tiles, va_tiles, qt_tiles = {}, {}, {}

            def load_g(g):
                kt = ktp.tile([P, SK], BF16, tag="kt", name=f"kt{g}")
                nc.sync.dma_start(kt[:], k_d[g])
                va = vap.tile([P, NKB, D + 1], BF16, tag="va", name=f"va{g}")
                nc.sync.dma_start(va[:], v_d[g])
                kt_tiles[g] = kt
                va_tiles[g] = va

            def load_h(hi):
                qt = qtp.tile([P, SQ], BF16, tag="qt", name=f"qt{hi}")
                nc.sync.dma_start(qt[:], q_d[hi])
                qt_tiles[hi] = qt

            # first head's tensors arrive in consumption order, chunked so the
            # first score matmul only waits for the first slices; issue
            # alternates between the SP and gpsimd DGEs so two rings fill in
            # parallel
            kt0 = ktp.tile([P, SK], BF16, tag="kt", name="kt0")
            va0 = vap.tile([P, NKB, D + 1], BF16, tag="va", name="va0")
            qt0 = qtp.tile([P, SQ], BF16, tag="qt", name="qt0")
            kt_tiles[0] = kt0
            va_tiles[0] = va0
            qt_tiles[0] = qt0

            def kchunk(lo, hi):
                nc.sync.dma_start(kt0[:, lo * P : hi * P], k_d[0, :, lo * P : hi * P])

            def qchunk(sb):
                s = slice(sb * SBW, (sb + 1) * SBW)
                nc.sync.dma_start(qt0[:, s], q_d[0, :, s])

            def vchunk(lo, hi):
                nc.sync.dma_start(va0[:, lo:hi], v_d[0, :, lo:hi])

            kchunk(0, 2)
            qchunk(0)
            vchunk(0, 2)
            kchunk(2, 4)
            qchunk(1)
            vchunk(2, 4)
            kchunk(4, 8)
            qchunk(2)
            vchunk(4, 8)
            kchunk(8, 12)
            qchunk(3)
            vchunk(8, 16)
            kchunk(12, 16)

            pt_tiles = {}  # task idx -> pt AP
            pv_tiles = {}  # (hi, sb) -> [4 pv APs]
            ot_tiles = {}  # (hi, sb) -> staging AP

            def emit_s(t):
                hi, sb, p = tasks[t]
                g = heads[hi][0]
                r = 2 * p - 4 * sb
                st = stp.tile([P, 2, SBW], F32, tag="st", name=f"st{t}")
                for half in (0, 1):
                    kb = 2 * p + half
                    q_lo = max(0, kb - 4 * sb) * P
                    if t < 2:
                        q_lo = 0  # initialize the full PSUM slot on first use
                    nc.tensor.matmul(
                        st[:, half, q_lo:],
                        kt_tiles[g][:, kb * P : (kb + 1) * P],
                        qt_tiles[hi][:, sb * SBW + q_lo : (sb + 1) * SBW],
                        start=True,
                        stop=True,
                    )
                q_lo = max(0, r) * P
                pt = ptp.tile([P, 2, SBW], BF16, tag="pt", name=f"pt{t}")
                nc.scalar.activation(
                    pt[:, :, q_lo:],
                    st[:, :, q_lo:],
                    mybir.ActivationFunctionType.Exp,
                    scale=SCALE,
                )
                if r in (0, 2):
                    nc.vector.tensor_tensor(
                        out=pt[:, :, q_lo:],
                        in0=pt[:, :, q_lo:],
                        in1=msk[:, r // 2, :, q_lo:],
                        op=mybir.AluOpType.mult,
                    )
                pt_tiles[t] = pt

            def emit_pv(t):
                hi, sb, p = tasks[t]
                g = heads[hi][0]
                va = va_tiles[g]
                pt = pt_tiles.pop(t)
                if p == 0:
                    pv_tiles[(hi, sb)] = [
                        ppvp.tile([P, D + 1], F32, tag="pv", name=f"pv{hi}_{sb}_{j}")
                        for j in range(4)
                    ]
                    ot_tiles[(hi, sb)] = outp.tile(
                        [P, 4, D + 1], F32, tag="ot", name=f"ot{hi}_{sb}"
                    )
                pvs = pv_tiles[(hi, sb)]
                ot = ot_tiles[(hi, sb)]
                for half in (0, 1):
                    kb = 2 * p + half
                    for j in range(max(0, kb - 4 * sb), 4):
                        qb = 4 * sb + j
                        nc.tensor.matmul(
                            pvs[j][:],
                            pt[:, half, j * P : (j + 1) * P],
                            va[:, kb, :],
                            start=(kb == 0),
                            stop=(kb == qb),
                        )
                        if kb == qb:
                            nc.vector.tensor_copy(ot[:, j], pvs[j][:])
                if p == 2 * sb + 1:
                    nc.sync.dma_start(
                        o_d[hi, sb],
                        ot[:].rearrange("p j d -> p (j d)"),
                    )
                    del ot_tiles[(hi, sb)]
                    del pv_tiles[(hi, sb)]

            head_first_task = {}
            for t, (hi, sb, p) in enumerate(tasks):
                if hi not in head_first_task:
                    head_first_task[hi] = t

            for t in range(len(tasks) + LOOKAHEAD):
                if t < len(tasks):
                    hi = tasks[t][0]
                    if head_first_task.get(hi) == t:
                        # prefetch next head's tensors while this head runs
                        if hi + 1 < NQH:
                            load_h(hi + 1)
                        if hi == 1:
                            load_g(1)
                    emit_s(t)
                if t >= LOOKAHEAD:
                    emit_pv(t - LOOKAHEAD)

    nc.finalize()
    return nc


def _get_module():
    if "nc" not in _CACHE:
        _CACHE["nc"] = build_module()
    return _CACHE["nc"]


def kernel(q, kv):
    global LAST_RESULTS
    q = np.asarray(q, dtype=np.float32)
    kv = np.asarray(kv, dtype=np.float32)

    nc = _get_module()

    in_maps = []
    for c in range(NCORES):
        b, j = divmod(c, 4)
        # q^T: [4 heads, 128 d, 2048 s]
        q_bf = q[b][:, 4 * j : 4 * j + 4, :].astype(BF16_NP)  # [S, 4, D]
        q_t = np.ascontiguousarray(np.transpose(q_bf, (1, 2, 0)))
        # k^T: [2 kv heads, 128 d, 2048 s]
        k_bf = kv[b][:, 0, 2 * j : 2 * j + 2, :].astype(BF16_NP)  # [S, 2, D]
        k_t = np.ascontiguousarray(np.transpose(k_bf, (1, 2, 0)))
        # v augmented with ones col: [2, 128 p, 16 kb, 129]
        v_bf = kv[b][:, 1, 2 * j : 2 * j + 2, :].astype(BF16_NP)  # [S, 2, D]
        v_a = np.ones((NKVH, P, NKB, D + 1), BF16_NP)
        v_a[:, :, :, :D] = np.transpose(
            v_bf.reshape(NKB, P, NKVH, D), (2, 1, 0, 3)
        )
        in_maps.append({"q": q_t, "k": k_t, "v": v_a})

    trace = bool(int(os.environ.get("KERNEL_TRACE", "0")))
    kwargs = {}
    tdir = os.environ.get("KERNEL_TRACE_DIR")
    if tdir:
        kwargs["tmpdir"] = tdir
    res = run_bass_kernel_spmd(
        nc, in_maps, core_ids=list(range(NCORES)), trace=trace, **kwargs
    )
    LAST_RESULTS = res

    out = np.empty((B, SQ, H, D), np.float32)
    for c in range(NCORES):
        b, j = divmod(c, 4)
        o = res.results[c]["o"].reshape(NQH, NSB, P, 4, D + 1)
        o = np.transpose(o, (0, 1, 3, 2, 4)).reshape(NQH, SQ, D + 1)
        norm = o[..., :D] / o[..., D : D + 1]
        out[b, :, 4 * j : 4 * j + 4, :] = np.transpose(norm, (1, 0, 2))
    return out


# revision 19
# speedup vs baseline: 1.1856x; 1.0020x over previous
"""Causal GQA cross-attention kernel for Trainium2, 8-core SPMD.

Problem: q [2, 2048, 16, 128] f32, kv [2, 2048, 2, 8, 128] f32 ->
out [2, 2048, 16, 128] f32; causal mask (Sq == Sk), GQA with 2 q heads
per kv head, softmax scale 1/sqrt(128).

Sharding: 2 batches x 4 kv-head-pairs -> 8 cores. Each core gets 4 q
heads + 2 kv heads (its GQA groups), computes attention locally; no
collectives. Host splits/gathers.

Per-core algorithm, all matmuls bf16 (host pre-rounds to bf16 and
pre-transposes, so the device does zero transposes/casts):
  - Q^T/K^T [128d, S] bf16 loaded with contiguous DMAs.
  - V pre-augmented on host with a ones column: [128p, 16kb, 129] bf16;
    the ones column yields the softmax denominator for free in PV.
  - Work unit = (head, q-superblock sb of 512, k-block PAIR p covering
    kb = 2p, 2p+1) with 2p <= 4*sb+3 (block-causal). Per task:
      S^T[k, q] = (K^T kb-block).T @ Q^T[, sb]  -> PSUM [128, 2, 512],
      one matmul per half, q cols clipped to the causal range.
      P^T = exp(S^T * scale) in ONE activation over both halves
      (halves the scalar engine's per-instruction overhead); clipped to
      the pair's causal range; diagonal pairs masked multiplicatively
      on DVE with precomputed pair masks.
      PV: pv[j] += (P^T q-block j).T @ Vaug[kb]  (PSUM f32 accumulate).
  - Tasks are software-pipelined with lookahead 2: the tensor queue
    order is S(0) S(1) [S(2) PV(0)] [S(3) PV(1)] ... so exp on the
    scalar engine overlaps score/PV matmuls instead of stalling the PE.
  - Store unnormalized [q, 4*(129)] per (head, superblock); host
    divides by the denominator column and reorders.
"""

import math
import os
import sys

import ml_dtypes
import numpy as np

sys.path.insert(0, "/opt/trn_rl_repo")

import concourse.bass as bass  # noqa: E402
import concourse.mybir as mybir  # noqa: E402
import concourse.tile as tile  # noqa: E402
from concourse import bacc  # noqa: E402
from concourse.bass_utils import run_bass_kernel_spmd  # noqa: E402

B, SQ, SK, H, HKV, D = 2, 2048, 2048, 16, 8, 128
NCORES = 8
NQH = H * B // NCORES  # 4 q heads per core
NKVH = HKV * B // NCORES  # 2 kv heads per core
P = 128
NQB = SQ // P  # 16 q blocks of 128
NSB = 4  # q superblocks of 512
SBW = 512
NKB = SK // P  # 16 k blocks
SCALE = 1.0 / math.sqrt(D)
LOOKAHEAD = 2

F32 = mybir.dt.float32
BF16 = mybir.dt.bfloat16
BF16_NP = ml_dtypes.bfloat16

LAST_RESULTS = None
_CACHE = {}


def build_module():
    nc = bacc.Bacc(None, target_bir_lowering=False)

    q_d = nc.dram_tensor("q", [NQH, D, SQ], BF16, kind="ExternalInput")
    k_d = nc.dram_tensor("k", [NKVH, D, SK], BF16, kind="ExternalInput")
    v_d = nc.dram_tensor("v", [NKVH, P, NKB, D + 1], BF16, kind="ExternalInput")
    # per (head, superblock): row p holds the 4 q-block outputs concatenated
    o_d = nc.dram_tensor("o", [NQH, NSB, P, 4 * (D + 1)], F32, kind="ExternalOutput")

    heads = [(g, hl) for g in range(NKVH) for hl in range(2)]
    # flat task list: (head index, q superblock, k-block pair)
    tasks = []
    for hi in range(NQH):
        for sb in range(NSB):
            for p in range(2 * sb + 2):
                tasks.append((hi, sb, p))

    with tile.TileContext(nc) as tc:
        with (
            tc.tile_pool(name="const", bufs=1) as constp,
            tc.tile_pool(name="kt", bufs=2) as ktp,
            tc.tile_pool(name="qt", bufs=2) as qtp,
            tc.tile_pool(name="vaug", bufs=2) as vap,
            tc.tile_pool(name="pt", bufs=4) as ptp,
            tc.tile_pool(name="outs", bufs=4) as outp,
            tc.tile_pool(name="st", bufs=2, space="PSUM") as stp,
            tc.tile_pool(name="ppv", bufs=4, space="PSUM") as ppvp,
        ):
            msk = constp.tile([P, 2, 2, SBW], BF16, tag="msk", name="msk")

            kt_tiles, va_tiles, qt_tiles = {}, {}, {}

            def load_g(g):
                kt = ktp.tile([P, SK], BF16, tag="kt", name=f"kt{g}")
                nc.sync.dma_start(kt[:], k_d[g])
                va = vap.tile([P, NKB, D + 1], BF16, tag="va", name=f"va{g}")
                nc.sync.dma_start(va[:], v_d[g])
                kt_tiles[g] = kt
                va_tiles[g] = va

            def load_h(hi):
                qt = qtp.tile([P, SQ], BF16, tag="qt", name=f"qt{hi}")
                nc.sync.dma_start(qt[:], q_d[hi])
                qt_tiles[hi] = qt

            # first head's tensors arrive in consumption order, chunked so the
            # first score matmul only waits for the first slices; issue
            # alternates between the SP and gpsimd DGEs so two rings fill in
            # parallel
            kt0 = ktp.tile([P, SK], BF16, tag="kt", name="kt0")
            va0 = vap.tile([P, NKB, D + 1], BF16, tag="va", name="va0")
            qt0 = qtp.tile([P, SQ], BF16, tag="qt", name="qt0")
            kt_tiles[0] = kt0
            va_tiles[0] = va0
            qt_tiles[0] = qt0

            def kchunk(lo, hi):
                nc.sync.dma_start(kt0[:, lo * P : hi * P], k_d[0, :, lo * P : hi * P])

            def qchunk(sb):
                s = slice(sb * SBW, (sb + 1) * SBW)
                nc.sync.dma_start(qt0[:, s], q_d[0, :, s])

            def vchunk(lo, hi):
                nc.sync.dma_start(va0[:, lo:hi], v_d[0, :, lo:hi])

            # gpsimd's DGE exits the preamble earlier than SP's: issue the
            # first q/v chunks there while SP loads k, then build the pair
            # masks on gpsimd (needed only after the first activation)
            s0 = slice(0, SBW)
            nc.gpsimd.dma_start(qt0[:, s0], q_d[0, :, s0])
            nc.gpsimd.dma_start(va0[:, 0:2], v_d[0, :, 0:2])
            kchunk(0, 2)
            nc.gpsimd.memset(msk[:], 1.0)
            kchunk(2, 4)
            for i, r0 in enumerate((0, 2)):
                nc.gpsimd.affine_select(
                    out=msk[:, i],
                    in_=msk[:, i],
                    compare_op=mybir.AluOpType.is_ge,
                    fill=0.0,
                    base=-P * r0,
                    pattern=[[-P, 2], [1, SBW]],
                    channel_multiplier=-1,
                )
            qchunk(1)
            vchunk(2, 4)
            kchunk(4, 8)
            qchunk(2)
            vchunk(4, 8)
            kchunk(8, 12)
            qchunk(3)
            vchunk(8, 16)
            kchunk(12, 16)

            pt_tiles = {}  # task idx -> pt AP
            pv_tiles = {}  # (hi, sb) -> [4 pv APs]
            ot_tiles = {}  # (hi, sb) -> staging AP

            def emit_s(t):
                hi, sb, p = tasks[t]
                g = heads[hi][0]
                r = 2 * p - 4 * sb
                st = stp.tile([P, 2, SBW], F32, tag="st", name=f"st{t}")
                for half in (0, 1):
                    kb = 2 * p + half
                    q_lo = max(0, kb - 4 * sb) * P
                    if t < 2:
                        q_lo = 0  # initialize the full PSUM slot on first use
                    nc.tensor.matmul(
                        st[:, half, q_lo:],
                        kt_tiles[g][:, kb * P : (kb + 1) * P],
                        qt_tiles[hi][:, sb * SBW + q_lo : (sb + 1) * SBW],
                        start=True,
                        stop=True,
                    )
                q_lo = max(0, r) * P
                pt = ptp.tile([P, 2, SBW], BF16, tag="pt", name=f"pt{t}")
                nc.scalar.activation(
                    pt[:, :, q_lo:],
                    st[:, :, q_lo:],
                    mybir.ActivationFunctionType.Exp,
                    scale=SCALE,
                )
                if r in (0, 2):
                    nc.vector.tensor_tensor(
                        out=pt[:, :, q_lo:],
                        in0=pt[:, :, q_lo:],
                        in1=msk[:, r // 2, :, q_lo:],
                        op=mybir.AluOpType.mult,
                    )
                pt_tiles[t] = pt

            def emit_pv(t):
                hi, sb, p = tasks[t]
                g = heads[hi][0]
                va = va_tiles[g]
                pt = pt_tiles.pop(t)
                if p == 0:
                    pv_tiles[(hi, sb)] = [
                        ppvp.tile([P, D + 1], F32, tag="pv", name=f"pv{hi}_{sb}_{j}")
                        for j in range(4)
                    ]
                    ot_tiles[(hi, sb)] = outp.tile(
                        [P, 4, D + 1], F32, tag="ot", name=f"ot{hi}_{sb}"
                    )
                pvs = pv_tiles[(hi, sb)]
                ot = ot_tiles[(hi, sb)]
                for half in (0, 1):
                    kb = 2 * p + half
                    for j in range(max(0, kb - 4 * sb), 4):
                        qb = 4 * sb + j
                        nc.tensor.matmul(
                            pvs[j][:],
                            pt[:, half, j * P : (j + 1) * P],
                            va[:, kb, :],
                            start=(kb == 0),
                            stop=(kb == qb),
                        )
                        if kb == qb:
                            nc.vector.tensor_copy(ot[:, j], pvs[j][:])
                if p == 2 * sb + 1:
                    nc.sync.dma_start(
                        o_d[hi, sb],
                        ot[:].rearrange("p j d -> p (j d)"),
                    )
                    del ot_tiles[(hi, sb)]
                    del pv_tiles[(hi, sb)]

            head_first_task = {}
            for t, (hi, sb, p) in enumerate(tasks):
                if hi not in head_first_task:
                    head_first_task[hi] = t

            for t in range(len(tasks) + LOOKAHEAD):
                if t < len(tasks):
                    hi = tasks[t][0]
                    if head_first_task.get(hi) == t:
                        # prefetch next head's tensors while this head runs
                        if hi + 1 < NQH:
                            load_h(hi + 1)
                        if hi == 1:
                            load_g(1)
                    emit_s(t)
                if t >= LOOKAHEAD:
                    emit_pv(t - LOOKAHEAD)

    nc.finalize()
    return nc


def _get_module():
    if "nc" not in _CACHE:
        _CACHE["nc"] = build_module()
    return _CACHE["nc"]


def kernel(q, kv):
    global LAST_RESULTS
    q = np.asarray(q, dtype=np.float32)
    kv = np.asarray(kv, dtype=np.float32)

    nc = _get_module()

    in_maps = []
    for c in range(NCORES):
        b, j = divmod(c, 4)
        # q^T: [4 heads, 128 d, 2048 s]
        q_bf = q[b][:, 4 * j : 4 * j + 4, :].astype(BF16_NP)  # [S, 4, D]
        q_t = np.ascontiguousarray(np.transpose(q_bf, (1, 2, 0)))
        # k^T: [2 kv heads, 128 d, 2048 s]
        k_bf = kv[b][:, 0, 2 * j : 2 * j + 2, :].astype(BF16_NP)  # [S, 2, D]
        k_t = np.ascontiguousarray(np.transpose(k_bf, (1, 2, 0)))
        # v augmented with ones col: [2, 128 p, 16 kb, 129]
        v_bf = kv[b][:, 1, 2 * j : 2 * j + 2, :].astype(BF16_NP)  # [S, 2, D]
        v_a = np.ones((NKVH, P, NKB, D + 1), BF16_NP)
        v_a[:, :, :, :D] = np.transpose(
            v_bf.reshape(NKB, P, NKVH, D), (2, 1, 0, 3)
        )
        in_maps.append({"q": q_t, "k": k_t, "v": v_a})

    trace = bool(int(os.environ.get("KERNEL_TRACE", "0")))
    kwargs = {}
    tdir = os.environ.get("KERNEL_TRACE_DIR")
    if tdir:
        kwargs["tmpdir"] = tdir
    res = run_bass_kernel_spmd(
        nc, in_maps, core_ids=list(range(NCORES)), trace=trace, **kwargs
    )
    LAST_RESULTS = res

    out = np.empty((B, SQ, H, D), np.float32)
    for c in range(NCORES):
        b, j = divmod(c, 4)
        o = res.results[c]["o"].reshape(NQH, NSB, P, 4, D + 1)
        o = np.transpose(o, (0, 1, 3, 2, 4)).reshape(NQH, SQ, D + 1)
        norm = o[..., :D] / o[..., D : D + 1]
        out[b, :, 4 * j : 4 * j + 4, :] = np.transpose(norm, (1, 0, 2))
    return out
